# revision 128
# baseline (speedup 1.0000x reference)
"""Grouped-query attention (2 query heads, 1 pooled KV head) with RoPE,
causal softmax — Trainium2 Bass/Tile kernel, 8 NeuronCores.

Sharding: one core per (batch, head) pair (4 x 2 = 8 cores). The pooled KV
head is head-averaged on the host (mean over heads commutes with the linear
projection), so each core does: q/k/v projections, RoPE on q/k, causal
attention.

v12 strategy (fp8 DoubleRow hi/lo everywhere; 242us -> 173.9us modeled+HW):
  - EVERY matmul is an fp8e4m3 DoubleRow matmul (0.5 PE cycles/row,
    256-deep contraction per instruction = 4x bf16 FLOP rate). Full
    precision is kept with a hi+lo decomposition: a = fp8(a) + fp8(a -
    fp8(a)), and a.b is computed as ah.bh + al.bh + ah.bl (3 DoubleRow
    terms per 2 contraction blocks = 0.75 cycles/row at ~7-bit-mantissa
    accuracy, slightly BETTER than bf16's 1.0 cycles/row). PSUM stays f32.
    The q (k) projection spends error budget for speed: it keeps a
    residual term on only 2/8 (5/8) c-pairs; q drops the W-side residual
    (not x's), so wq's lo plane is 4 c-planes instead of 16 — 0.875MB off
    the startup DMA stream that gates wk. Measured end-to-end rel err
    1.743e-2 vs the 2e-2 gate (deterministic seed-0 inputs; the numpy
    error sim in sim_err.py predicts HW error to ~0.3% relative and is
    the tuning instrument for these drops).
  - subnormal guard: e4m3's smallest subnormal is 2^-9, so small-magnitude
    tensors are pre-scaled by a power of 2 before splitting (weights x64 on
    the host, v x4 on device); the descale folds into the rope tables, the
    v copies, and the host's final division. x/w hi+lo pairs are split on
    the host and DMA'd interleaved (same bytes as bf16).
  - q/k are projected directly into transposed [d, t] layout (stationary =
    W^T chunk, moving = xT chunk); RoPE (fp16 tables, DVE) writes fp8
    {hi,lo} pairs: ScalarE casts hi, GpSimd subtracts the lo residual
    (off the critical path). v in natural [t, d]; per-256-block
    evens-then-odds (pi2) d-permutation makes RoPE a half-block swap and
    cancels in the q.kT contraction.
  - attention: scoresT [s, t] via 6 DoubleRow matmuls per s-tile (2 d-pairs
    x 3 hi/lo terms); exp on ScalarE to an f32 tile; tri-mask on DVE; the
    {eh, el} fp8 splits live in [P, 2, TS] s-PAIR tiles (ScalarE/DVE
    alternate the hi cast; DVE subtracts lo) so the AV DoubleRow contracts
    2 s-tiles at once: 12 matmuls + 2 ones-vector sums matmuls per pair
    (denominators accumulate in a [32, TS] PSUM bank; no GpSimd reduce).
  - software pipeline by s-tile pairs, depth 2, with pending AV groups
    riding ACROSS slab boundaries (stale-head pop rule): the old slab's
    drain copies always run >=2 s-tiles ahead of the new slab's start=True
    AV write, and the exp->split latency is always covered by score matmuls.
  - diagonal s-tiles narrow their operands to t >= m*128 (causal trim);
    only the 128-wide triangular block is masked; the odd member of a diag
    pair gets its dead strip memset to zero.
  - normalization (1/(4*sum)) happens on the HOST: the device ships the raw
    AV accumulator (bf16) and the per-column sums; the final pair's drain is
    interleaved per d-half with its own AV matmuls to shorten the tail.
  - startup: DMA stream ordered by first consumption (wq/x halves, fp16
    tables, wk/wv halves); a 15-matmul warm-up train holds the PE p-state
    through the DMA wait; slab-0's qp PSUM bounces through SBUF via ScalarE
    so the k-pass isn't gated on rope-table arrival.
"""

import sys

sys.path.insert(0, "/opt/trn_rl_repo")

import numpy as np

B, T, C = 4, 2048, 2048
H, D = 2, 512
NCORES = 8
ROPE_THETA = 10000.0
P = 128
NT = T // P  # 16 t/s tiles of 128
NCC = C // P  # 16 contraction chunks of 128
NDT = D // P  # 4 head-dim tiles of 128
NFT = D // 2 // P  # 2 freq tiles of 128
TS = 512  # t-slab width
NSLAB = T // TS  # 4
NST = TS // P  # 4 s-tiles per slab width
SW = 64.0  # weight pre-scale for fp8 (power of 2; exact)
ISW = 1.0 / SW

_CACHE = {}


def _emit_consumers(nc, DR, vh_sb, vl_sb, ones8, pend):
    """AV DoubleRow matmuls for a pipelined s-tile PAIR: stationary is the
    v pair [s2, d-tile], moving the {eh,el} pair tiles. 3 hi/lo terms per
    d-tile (vh.eh + vl.eh + vh.el). The softmax denominators ride along as
    two ones-vector DoubleRow matmuls into a [1, TS] PSUM accumulator
    (~107ns each) — cheaper than a GpSimd partition reduce + DVE
    accumulation, and off the PE's dependency-critical DVE queue."""
    j, av, sacc, ehp, elp, lo, u, start, stop = pend
    for dt in range(NDT):
        dsl = slice(dt * P, (dt + 1) * P)
        for vt, e, ti in ((vh_sb, ehp, 0), (vl_sb, ehp, 1), (vh_sb, elp, 2)):
            nc.tensor.matmul(
                av[dt][:, lo:TS],
                vt[:, 2 * u : 2 * u + 2, dsl],
                e[:, :, lo:TS],
                start=(start and ti == 0),
                stop=(stop and ti == 2),
                perf_mode=DR,
            )
    for e, ti in ((ehp, 0), (elp, 1)):
        nc.tensor.matmul(
            sacc[:, lo:TS],
            ones8,
            e[:, :, lo:TS],
            start=(start and ti == 0),
            stop=(stop and ti == 1),
            perf_mode=DR,
        )


def _build():
    import concourse.bass as bass
    import concourse.mybir as mybir
    from concourse import bacc
    from concourse.tile import TileContext

    fp32 = mybir.dt.float32
    fp16 = mybir.dt.float16
    bf16 = mybir.dt.bfloat16
    fp8 = mybir.dt.float8e4
    EXP = mybir.ActivationFunctionType.Exp
    DR = mybir.MatmulPerfMode.DoubleRow

    nc = bacc.Bacc()
    # x and weights ship as interleaved fp8 hi/lo pairs: [c, {hi,lo}, free].
    # Weights are pre-scaled by SW (power of 2) on the host so their residual
    # (lo) parts clear the e4m3 subnormal floor; the descale folds into the
    # rope tables (q/k) and the v PSUM->SBUF copies.
    # x is slab-major so a slab's {hi,lo} pair is one contiguous DMA block
    xT = nc.dram_tensor("xT", [C, NSLAB, 2, TS], fp8, kind="ExternalInput")
    wq = nc.dram_tensor("wq", [C, 2, D], fp8, kind="ExternalInput")  # pi2-perm
    wk = nc.dram_tensor("wk", [C, 2, D], fp8, kind="ExternalInput")  # pi2, avg
    wv = nc.dram_tensor("wv", [C, 2, D], fp8, kind="ExternalInput")  # head-avg
    # rope tables in fp16 (values <= 1/SW; rel err ~2e-4): halves the table
    # DMA so wk lands before the k-pass needs it
    cosa = nc.dram_tensor("cosa", [P, NFT, T], fp16, kind="ExternalInput")
    sina = nc.dram_tensor("sina", [P, NFT, T], fp16, kind="ExternalInput")
    tri = nc.dram_tensor("tri", [P, P], bf16, kind="ExternalInput")
    oT = nc.dram_tensor("oT", [D, T], bf16, kind="ExternalOutput")
    sums_d = nc.dram_tensor("sums_d", [NSLAB, TS], fp32, kind="ExternalOutput")

    scale = float(D) ** -0.5

    with TileContext(nc) as tc:
        with (
            tc.tile_pool(name="persist", bufs=1) as pp,
            tc.tile_pool(name="psum", bufs=1, space="PSUM") as ps,
        ):
            # q/k/v persist as fp8 hi/lo pairs (same SBUF bytes as bf16)
            qh_sb = pp.tile([P, NDT, T], fp8)  # [d, t], pi2 layout
            ql_sb = pp.tile([P, NDT, T], fp8)
            kh_sb = pp.tile([P, NDT, T], fp8)
            kl_sb = pp.tile([P, NDT, T], fp8)
            vh_sb = pp.tile([P, NT, D], fp8)  # [t, d], natural, 4x scaled
            vl_sb = pp.tile([P, NT, D], fp8)
            tri_sb = pp.tile([P, P], bf16)
            ones_sb = pp.tile([P, 1], bf16)
            nc.vector.memset(ones_sb, 1.0)
            # DoubleRow ones for the denominators. 32 stationary columns (the
            # minimum legal PE tile width); every output partition row gets
            # the same column sums and the drain reads row 0.
            ones8 = pp.tile([P, 2, 32], fp8)
            nc.vector.memset(ones8, 1.0)
            warm_sb = pp.tile([P, TS], bf16)
            nc.gpsimd.memset(warm_sb, 0.0)  # parallel with the DVE memsets
            # warm-up train: keeps the PE busy through the startup DMA wait
            # so the p-state ramp is done (and no idle gap resets it) by the
            # time the first projection matmul's data lands (~5.5us: wq[0:8]
            # + the first two xs0 quarters). Sized to end just then — longer
            # only burns PE time.
            wps = ps.tile([1, TS], fp32, tag="sc", bufs=3, name="wps")
            for _ in range(11):
                nc.tensor.matmul(wps, ones_sb, warm_sb, start=True, stop=True)

            # ---------------- phase 1: projections + rope -----------------
            with (
                tc.tile_pool(name="wpool", bufs=1) as wp,
                tc.tile_pool(name="xpool", bufs=2) as xp,
                tc.tile_pool(name="rope", bufs=3) as rp,
            ):
                wq_sb = wp.tile([P, NCC, 2, D], fp8)
                wk_sb = wp.tile([P, NCC, 2, D], fp8)
                wv_sb = wp.tile([P, NCC, 2, D], fp8)
                # DMA plan: the HWDGE is ONE shared device (~630ns/DMA), so
                # everything goes on the sync queue in strict need-order with
                # as few instructions as possible: wq whole (PE blocked on it
                # is covered by the warm-up train), slab-0 x in 4 quarters
                # streaming just ahead of the half-pass consumption, wk before
                # the rope tables (k-pass needs it ~23us in; rope is DVE-side
                # and can wait), wv, then whole-slab prefetches.
                wqr = wq.rearrange("(cc p) two d -> p cc two d", p=P)
                xs0 = xp.tile([P, NCC, 2, TS], fp8, tag="xs", name="xs0")
                xTr = xT.rearrange("(cc p) slab two t -> p cc slab two t", p=P)
                cs0 = xp.tile([P, NFT, TS], fp16, tag="cos", name="cs0")
                sn0 = xp.tile([P, NFT, TS], fp16, tag="sin", name="sn0")
                # every DMA costs ~630ns of HWDGE on top of bytes/BW, so the
                # startup stream uses as few, as-large DMAs as possible, each
                # completing just before the PE consumes it
                nc.sync.dma_start(out=wq_sb[:, 0:8, 0], in_=wqr[:, 0:8, 0])
                nc.sync.dma_start(out=xs0[:, 0:8], in_=xTr[:, 0:8, 0])
                nc.sync.dma_start(out=wq_sb[:, 8:16, 0], in_=wqr[:, 8:16, 0])
                nc.sync.dma_start(out=xs0[:, 8:12], in_=xTr[:, 8:12, 0])
                nc.sync.dma_start(out=xs0[:, 12:16], in_=xTr[:, 12:16, 0])
                nc.sync.dma_start(out=wq_sb[:, 0:4, 1], in_=wqr[:, 0:4, 1])
                nc.sync.dma_start(out=cs0, in_=cosa[:, :, 0:TS])
                nc.sync.dma_start(out=sn0, in_=sina[:, :, 0:TS])
                # wk/wv in halves: the k-pass (v-pass) consumes c-planes in
                # order, so the first half's completion sem unblocks the PE
                # ~2.8us before the full tensor would
                wkr = wk.rearrange("(cc p) two d -> p cc two d", p=P)
                wvr = wv.rearrange("(cc p) two d -> p cc two d", p=P)
                nc.sync.dma_start(out=wk_sb[:, 0:8], in_=wkr[:, 0:8])
                nc.sync.dma_start(out=wk_sb[:, 8:16], in_=wkr[:, 8:16])
                nc.sync.dma_start(out=wv_sb[:, 0:8], in_=wvr[:, 0:8])
                nc.sync.dma_start(out=wv_sb[:, 8:16], in_=wvr[:, 8:16])
                nc.sync.dma_start(out=tri_sb, in_=tri[:, :])

                for j in range(NSLAB):
                    tsl = slice(j * TS, (j + 1) * TS)
                    if j == 0:
                        xs, cs, sn = xs0, cs0, sn0
                    else:
                        xs = xp.tile([P, NCC, 2, TS], fp8, tag="xs", name="xs")
                        nc.sync.dma_start(out=xs, in_=xTr[:, :, j])
                        cs = xp.tile([P, NFT, TS], fp16, tag="cos", name="cs")
                        sn = xp.tile([P, NFT, TS], fp16, tag="sin", name="sn")
                        nc.sync.dma_start(out=cs, in_=cosa[:, :, tsl])
                        nc.sync.dma_start(out=sn, in_=sina[:, :, tsl])

                    # q / k half-passes: project 2 d-tiles into [d, t], RoPE.
                    # Each c-PAIR contracts 256 rows per DoubleRow matmul; the
                    # hi/lo decomposition needs 3 terms (wh.xh + wl.xh + wh.xl)
                    # per pair = 0.75 PE cycles/row vs bf16's 1.0, at ~7-bit
                    # mantissa accuracy (better than bf16).
                    # nxl = how many leading c-pairs keep the (wh, xl)
                    # residual term (q 2/8, k 6/8): the error feeds only the
                    # softmax scores where the budget is cheapest; measured
                    # end-to-end rel err 1.698e-2 vs the 2e-2 gate
                    # (deterministic seed-0 inputs).
                    def qk_pass(w_sb, dsth, dstl, hi_copy, nxl, flip=False):
                        for h2 in range(2):  # freq-group half (d-tiles 2h2, 2h2+1)
                            qp = ps.tile([P, 2, TS], fp32, tag="pp", bufs=2, name="qp")
                            if flip:
                                # q: the dropped residual is w-side, so wq's
                                # lo plane is only needed for nxl c-pairs —
                                # 0.875MB off the startup DMA stream that
                                # gates wk. The (wl, xh) tail runs last so
                                # the small late wl DMA never stalls the PE.
                                for cp in range(NCC // 2):
                                    c2 = slice(2 * cp, 2 * cp + 2)
                                    for i in range(2):
                                        dt = slice(
                                            (2 * h2 + i) * P, (2 * h2 + i + 1) * P
                                        )
                                        for ti, xi in enumerate((0, 1)):
                                            nc.tensor.matmul(
                                                qp[:, i, :],
                                                w_sb[:, c2, 0, dt],
                                                xs[:, c2, xi, :],
                                                start=(cp == 0 and ti == 0),
                                                stop=False,
                                                perf_mode=DR,
                                            )
                                for cp in range(nxl):
                                    c2 = slice(2 * cp, 2 * cp + 2)
                                    for i in range(2):
                                        dt = slice(
                                            (2 * h2 + i) * P, (2 * h2 + i + 1) * P
                                        )
                                        nc.tensor.matmul(
                                            qp[:, i, :],
                                            w_sb[:, c2, 1, dt],
                                            xs[:, c2, 0, :],
                                            start=False,
                                            stop=(cp == nxl - 1),
                                            perf_mode=DR,
                                        )
                            else:
                                for cp in range(NCC // 2):
                                    c2 = slice(2 * cp, 2 * cp + 2)
                                    terms = (
                                        ((0, 0), (1, 0), (0, 1))
                                        if cp < nxl
                                        else ((0, 0), (1, 0))
                                    )
                                    for i in range(2):
                                        dt = slice(
                                            (2 * h2 + i) * P, (2 * h2 + i + 1) * P
                                        )
                                        for ti, (wi, xi) in enumerate(terms):
                                            nc.tensor.matmul(
                                                qp[:, i, :],
                                                w_sb[:, c2, wi, dt],
                                                xs[:, c2, xi, :],
                                                start=(cp == 0 and ti == 0),
                                                stop=(
                                                    cp == NCC // 2 - 1
                                                    and ti == len(terms) - 1
                                                ),
                                                perf_mode=DR,
                                            )
                            # slab 0: bounce qp to SBUF on the (startup-idle)
                            # ScalarE so the PSUM slot frees immediately —
                            # the RoPE muls otherwise hold it hostage to the
                            # rope-table DMAs, stalling the k-pass behind
                            # the startup DMA stream.
                            if j == 0:
                                qpc = rp.tile(
                                    [P, 2, TS], fp32, tag="qpc", bufs=2, name="qpc"
                                )
                                nc.scalar.copy(qpc[:, 0, :], qp[:, 0, :])
                                nc.scalar.copy(qpc[:, 1, :], qp[:, 1, :])
                                qp = qpc
                            # RoPE within the half: partner = other i, ft = h2
                            m = [None, None, None, None]
                            for i in range(2):
                                m[2 * i] = rp.tile([P, TS], fp32, tag=f"m{2 * i}", name="m1")
                                m[2 * i + 1] = rp.tile(
                                    [P, TS], fp32, tag=f"m{2 * i + 1}", name="m2"
                                )
                                nc.vector.tensor_mul(
                                    m[2 * i], qp[:, i, :], cs[:, h2, :]
                                )
                                nc.vector.tensor_mul(
                                    m[2 * i + 1], qp[:, 1 - i, :], sn[:, h2, :]
                                )
                            # t0' = t0*cos - t1*sin ; t1' = t1*cos + t0*sin,
                            # then split into fp8 hi (ScalarE cast) + residual
                            # lo. The residual sub runs on the otherwise-idle
                            # GpSimd: it's slow there (~1us) but gates nothing
                            # until this slab's column-window of attention,
                            # and it keeps DVE's queue short for the splits
                            # the AV matmuls actually wait on.
                            for dd, ma, mb, addsub in (
                                (2 * h2, m[0], m[1], nc.vector.tensor_sub),
                                (2 * h2 + 1, m[2], m[3], nc.vector.tensor_add),
                            ):
                                tt = rp.tile([P, TS], fp32, tag=f"t{dd % 2}", name="tt")
                                addsub(tt, ma, mb)
                                hi_copy(dsth[:, dd, tsl], tt)
                                nc.gpsimd.tensor_sub(
                                    dstl[:, dd, tsl], tt, dsth[:, dd, tsl]
                                )

                    # v half-passes: natural [t, d] layout (stationary = x)
                    def v_pass():
                        for h2 in range(2):
                            vp = ps.tile([P, 2, D], fp32, tag="pp", bufs=2, name="vp")
                            for cp in range(NCC // 2):
                                c2 = slice(2 * cp, 2 * cp + 2)
                                for i in range(2):
                                    tt = slice((2 * h2 + i) * P, (2 * h2 + i + 1) * P)
                                    for xi, wi, ti in ((0, 0, 0), (1, 0, 1), (0, 1, 2)):
                                        nc.tensor.matmul(
                                            vp[:, i, :],
                                            xs[:, c2, xi, tt],
                                            wv_sb[:, c2, wi, :],
                                            start=(cp == 0 and ti == 0),
                                            stop=(cp == NCC // 2 - 1 and ti == 2),
                                            perf_mode=DR,
                                        )
                            # v split: hi = fp8(4v) via ScalarE scaled cast,
                            # lo = 4v - hi in ONE DVE scalar_tensor_tensor.
                            # The 4x scale keeps v residuals clear of the e4m3
                            # subnormal floor; the host divides av by 4.
                            for i in range(2):
                                tv = j * NDT + 2 * h2 + i
                                nc.scalar.mul(vh_sb[:, tv, :], vp[:, i, :], 4.0 * ISW)
                                nc.vector.scalar_tensor_tensor(
                                    out=vl_sb[:, tv, :],
                                    in0=vp[:, i, :],
                                    scalar=4.0 * ISW,
                                    in1=vh_sb[:, tv, :],
                                    op0=mybir.AluOpType.mult,
                                    op1=mybir.AluOpType.subtract,
                                )

                    # k's hi casts go on GpSimd: on ACT they'd head-of-line
                    # block (waiting their DVE-produced inputs) the v-split
                    # muls and the first attention exp behind them at the
                    # phase boundary; nothing reads kh until this slab's own
                    # attention column-window, so GpSimd's ~1us/op is fine.
                    qk_pass(wq_sb, qh_sb, ql_sb, nc.scalar.copy, 2, flip=True)
                    qk_pass(wk_sb, kh_sb, kl_sb, nc.gpsimd.tensor_copy, 5)
                    v_pass()

            # ---------------- phase 2: causal attention -------------------
            with (
                tc.tile_pool(name="expp", bufs=3) as ep,
                tc.tile_pool(name="outp", bufs=4) as op_,
            ):
                oTr = oT.rearrange("(g p) t -> p g t", p=P)

                def drain(j, sacc, av):
                    # ob DMAs go on the queue BEFORE the sums DMA so the
                    # small sums transfer can't head-block them; sums ships
                    # straight from its PSUM accumulator.
                    for g in range(2):  # paired output: 2 DMAs instead of 4
                        ob = op_.tile([P, 2, TS], bf16, tag="ob", name="ob")
                        nc.scalar.copy(ob[:, 0, :], av[2 * g])
                        nc.vector.tensor_copy(ob[:, 1, :], av[2 * g + 1])
                        nc.sync.dma_start(
                            out=oTr[:, 2 * g : 2 * g + 2, j * TS : (j + 1) * TS],
                            in_=ob,
                        )
                    sb = op_.tile([1, TS], fp32, tag="acc", name="sb")
                    nc.scalar.copy(sb, sacc[0:1, :])
                    nc.sync.dma_start(out=sums_d[j, :], in_=sb)

                # Software-pipelined by s-tile PAIRS (the AV DoubleRow
                # contraction spans 2 s-tiles): scores for later tiles are
                # emitted BEFORE a pair's AV consumers, covering the
                # exp->split round-trip latency — a >100ns PE gap costs a
                # ~3us half-clock p-state re-ramp, so every gap matters. The
                # softmax denominators run entirely off the PE: the exact f32
                # exp tile feeds a GpSimd partition reduce, DVE accumulates
                # row 0. The {eh,el} fp8 splits live in [P, 2, TS] pair tiles
                # so the AV moving operand is a contiguous s-pair.
                pends = []  # of (j, av, acc, ehp, elp, lo, u, start, stop)
                from concourse import bass_isa

                for j in range(NSLAB):
                    nst = NST * (j + 1)  # s-tiles needed (causal)
                    sacc = ps.tile([32, TS], fp32, tag="sums", bufs=1, name="sacc")
                    ava = ps.tile([P, 2, TS], fp32, tag="pp", bufs=2, name="ava")
                    avb = ps.tile([P, 2, TS], fp32, tag="pp", bufs=2, name="avb")
                    av = [ava[:, 0, :], ava[:, 1, :], avb[:, 0, :], avb[:, 1, :]]
                    ehp = elp = None
                    pair_lo = 0
                    for st in range(nst):
                        mdiag = st - NST * j  # diagonal offset (>=0 on diag)
                        lo = mdiag * P if mdiag > 0 else 0
                        tslw = slice(j * TS + lo, (j + 1) * TS)
                        sc = ps.tile([P, TS], fp32, tag="sc", bufs=3, name="sc")
                        for dp in range(2):  # d-tile pairs, 3 hi/lo terms each
                            d2 = slice(2 * dp, 2 * dp + 2)
                            for kt, qt, ti in (
                                (kh_sb, qh_sb, 0),
                                (kl_sb, qh_sb, 1),
                                (kh_sb, ql_sb, 2),
                            ):
                                nc.tensor.matmul(
                                    sc[:, lo:TS],
                                    kt[:, d2, st * P : (st + 1) * P],
                                    qt[:, d2, tslw],
                                    start=(dp == 0 and ti == 0),
                                    stop=(dp == 1 and ti == 2),
                                    perf_mode=DR,
                                )
                        et = ep.tile([P, TS], fp32, tag="exf", bufs=4, name="et")
                        nc.scalar.activation(
                            out=et[:, lo:TS], in_=sc[:, lo:TS], func=EXP,
                            scale=scale,
                        )
                        if mdiag >= 0:  # diagonal: mask triangular 128-block
                            nc.vector.tensor_mul(
                                et[:, lo : lo + P], et[:, lo : lo + P], tri_sb
                            )
                        if st % 2 == 0:
                            ehp = ep.tile([P, 2, TS], fp8, tag="ehp", bufs=3, name="ehp")
                            elp = ep.tile([P, 2, TS], fp8, tag="elp", bufs=3, name="elp")
                            pair_lo = lo
                            if mdiag >= 0:
                                # diag pair: the odd tile only writes
                                # [lo+P, TS); zero its strip in the pair's
                                # shared column range [lo, lo+P)
                                nc.vector.memset(ehp[:, 1, lo : lo + P], 0.0)
                                nc.vector.memset(elp[:, 1, lo : lo + P], 0.0)
                        side = st % 2
                        # eh casts alternate DVE/ACT by tile parity (slab 0
                        # all-DVE): keeps ACT near exp-only so the sc PSUM
                        # rotation -> exp never paces the PE, without pushing
                        # DVE (el splits + drain copies) over its budget
                        if j == 0 or st % 2 == 1:
                            nc.vector.tensor_copy(ehp[:, side, lo:TS], et[:, lo:TS])
                        else:
                            nc.scalar.copy(ehp[:, side, lo:TS], et[:, lo:TS])
                        nc.vector.tensor_sub(
                            elp[:, side, lo:TS], et[:, lo:TS], ehp[:, side, lo:TS]
                        )
                        # pop when at depth 2 pairs, or when the head pend is
                        # from the PREVIOUS slab (st==0 case): the old slab's
                        # last AV pair + drain emit under this slab's first
                        # scores, and the lookahead puts ~4 s-tiles of PE work
                        # between the drain's PSUM->SBUF copies and this
                        # slab's start=True AV write.
                        if len(pends) == 2 or (pends and pends[0][0] != j):
                            pend = pends.pop(0)
                            _emit_consumers(nc, DR, vh_sb, vl_sb, ones8, pend)
                            if pend[8]:  # was its slab's last pair
                                drain(pend[0], pend[2], pend[1])
                        if st % 2 == 1:
                            u = st // 2
                            pends.append(
                                (j, av, sacc, ehp, elp, pair_lo, u,
                                 u == 0, u == nst // 2 - 1)
                            )
                    # Pending AV pairs ride across the slab boundary and pop
                    # via the stale-head rule during the next slab's first
                    # s-tiles: that buys the DVE queue (phase-1 RoPE tails,
                    # drain copies) 2+ extra s-tiles before anything on the
                    # PE waits for it. The last slab falls through with its
                    # final two pairs pending for the split tail drain.

                # tail: flush the second-to-last pair, then the last pair
                # with the drain interleaved per d-half — ob0's copies/DMA
                # start as soon as av[0..1] take their stop matmuls (ava
                # banks), overlapping the av[2..3] (avb) matmuls, so the
                # kernel tail is one copy+DMA chain shorter.
                pend6 = pends.pop(0)
                _emit_consumers(nc, DR, vh_sb, vl_sb, ones8, pend6)
                (j, av, sacc, ehp, elp, lo, u, start, stop) = pends[0]
                assert stop and not start
                for dt in range(NDT):
                    dsl = slice(dt * P, (dt + 1) * P)
                    for vt, e, ti in (
                        (vh_sb, ehp, 0), (vl_sb, ehp, 1), (vh_sb, elp, 2)
                    ):
                        nc.tensor.matmul(
                            av[dt][:, lo:TS],
                            vt[:, 2 * u : 2 * u + 2, dsl],
                            e[:, :, lo:TS],
                            start=False,
                            stop=(ti == 2),
                            perf_mode=DR,
                        )
                    if dt == 1:
                        ob0 = op_.tile([P, 2, TS], bf16, tag="ob", name="ob")
                        nc.scalar.copy(ob0[:, 0, :], av[0])
                        nc.vector.tensor_copy(ob0[:, 1, :], av[1])
                        nc.sync.dma_start(
                            out=oTr[:, 0:2, j * TS : (j + 1) * TS], in_=ob0
                        )
                for e, ti in ((ehp, 0), (elp, 1)):
                    nc.tensor.matmul(
                        sacc[:, lo:TS],
                        ones8,
                        e[:, :, lo:TS],
                        start=False,
                        stop=(ti == 1),
                        perf_mode=DR,
                    )
                ob1 = op_.tile([P, 2, TS], bf16, tag="ob", name="ob")
                nc.scalar.copy(ob1[:, 0, :], av[2])
                nc.vector.tensor_copy(ob1[:, 1, :], av[3])
                nc.sync.dma_start(
                    out=oTr[:, 2:4, j * TS : (j + 1) * TS], in_=ob1
                )
                sb = op_.tile([1, TS], fp32, tag="acc", name="sb")
                nc.scalar.copy(sb, sacc[0:1, :])
                nc.sync.dma_start(out=sums_d[j, :], in_=sb)

    nc.finalize()
    return nc


def _host_inputs(x, Wq, Wk, Wv):
    import ml_dtypes

    bf16 = ml_dtypes.bfloat16
    fp8 = ml_dtypes.float8_e4m3
    f32 = np.float32

    def hi_lo(a):  # [c, free] f32 -> [c, 2, free] fp8 (hi, residual)
        h = a.astype(fp8)
        l = (a - h.astype(f32)).astype(fp8)
        return np.ascontiguousarray(np.stack([h, l], axis=1))

    # per-256-block evens-then-odds: RoPE pairs stay within each d half
    pi2 = np.concatenate(
        [
            np.arange(0, D // 2, 2),
            np.arange(1, D // 2, 2),
            np.arange(D // 2, D, 2),
            np.arange(D // 2 + 1, D, 2),
        ]
    )

    wk_avg = Wk.mean(axis=0)  # [D, C]
    wv_avg = Wv.mean(axis=0)
    wk_p = hi_lo(np.ascontiguousarray(wk_avg.T[:, pi2]).astype(f32) * f32(SW))
    wv_t = hi_lo(np.ascontiguousarray(wv_avg.T).astype(f32) * f32(SW))

    freqs = 1.0 / (ROPE_THETA ** (np.arange(0, D, 2, dtype=np.float64) / D))
    ang = np.arange(T, dtype=np.float64)[None, :] * freqs[:, None]  # [D/2, T]
    cosa = np.ascontiguousarray(
        (np.cos(ang) * ISW).reshape(NFT, P, T).transpose(1, 0, 2)
    ).astype(np.float16)
    sina = np.ascontiguousarray(
        (np.sin(ang) * ISW).reshape(NFT, P, T).transpose(1, 0, 2)
    ).astype(np.float16)

    tri = (np.arange(P)[:, None] <= np.arange(P)[None, :]).astype(bf16)

    shared = {
        "wk": wk_p,
        "wv": wv_t,
        "cosa": cosa,
        "sina": sina,
        "tri": tri,
    }
    def x_slabbed(xb):  # [C, T] f32 -> [C, NSLAB, 2, TS] fp8
        h = xb.astype(fp8)
        l = (xb - h.astype(f32)).astype(fp8)
        out = np.stack(
            [h.reshape(C, NSLAB, TS), l.reshape(C, NSLAB, TS)], axis=2
        )
        return np.ascontiguousarray(out)

    xT_b = [x_slabbed(np.ascontiguousarray(x[b].T).astype(f32)) for b in range(B)]
    wq_h = [
        hi_lo(np.ascontiguousarray(Wq[h].T[:, pi2]).astype(f32) * f32(SW))
        for h in range(H)
    ]
    in_maps = []
    for i in range(NCORES):
        b, h = i // H, i % H
        in_maps.append({"xT": xT_b[b], "wq": wq_h[h], **shared})
    return in_maps


def _run(x, Wq, Wk, Wv, trace=False):
    from concourse.bass_utils import run_bass_kernel_spmd

    if "nc" not in _CACHE:
        _CACHE["nc"] = _build()
    in_maps = _host_inputs(x, Wq, Wk, Wv)
    res = run_bass_kernel_spmd(
        _CACHE["nc"], in_maps, list(range(NCORES)), trace=trace
    )
    out = np.empty((B, H, T, D), np.float32)
    for i in range(NCORES):
        r = res.results[i]
        o = r["oT"].astype(np.float32).T  # [T, D], unnormalized, 4x v-scale
        s = r["sums_d"].reshape(T)
        out[i // H, i % H] = o / (4.0 * s[:, None])
    return out.reshape(B, T, H * D), res


def kernel(**inputs):
    out, _ = _run(inputs["x"], inputs["Wq"], inputs["Wk"], inputs["Wv"])
    return out

